# revision 12
# baseline (speedup 1.0000x reference)
"""DigitalCapsule dynamic-routing kernel for 8 TRN2 NeuronCores.

Math (per batch b, out-capsule n):
    u_hat[p,d] = sum_e x[b,p,e] W[n,p,e,d]
    3 routing iters: c = softmax_p(logits), s = sum_p c*u_hat,
    v = squash(s), logits += v . u_hat
Output v: [B, N, D],  B=128, N=32, P=1152, E=8, D=16.

Sharding: N across the 8 cores (4 capsules each), full B per core.
Routing is independent per n, so there is no cross-core communication.

Formulation avoids materializing u_hat (302 MB) entirely:
  logits:  G[pe,(n,b)] = sum_d W[pe,d] vsum[n,d,b]   (fp16 matmul)
           aT[p,(n,b)] = sum_e xT[pe,b] * G[pe,(n,b)] (fp16 mul on DVE,
                          e-sum via identity matmuls into fp32 PSUM)
           where vsum accumulates v over iters, so logits never need
           to be carried: logits_3 = x.W.(v1+v2).
  s-step:  Y[pe,(n,b)] = exp(aT)*xT (bf16);  s = sum_pe Y W  (one
           free-512 matmul stream covers all 4 capsules).
All contractions run on the tensor engine; PSUM accumulates in fp32.
v lives in [d, b] layout throughout - no transposes anywhere.
"""

import numpy as np
import ml_dtypes

B, N, P, E, D = 128, 32, 1152, 8, 16
NCORES = 8
NS = N // NCORES          # capsules per core
PEF = P * E               # 9216 flattened (e, p) contraction dim
T = PEF // 128            # 72 K-tiles
PC = P // 128             # 9 p-chunks
EPS = 1e-8

_COMPILED = None


def _build():
    import concourse.bass as bass
    import concourse.tile as tile
    from concourse import bacc, mybir

    nc = bacc.Bacc("TRN2", target_bir_lowering=False)
    f16, bf16 = mybir.dt.float16, mybir.dt.bfloat16

    dram = {
        "xTh": nc.dram_tensor("xTh", [128, T, 128], f16, kind="ExternalInput"),
        "xTb": nc.dram_tensor("xTb", [128, T, 128], bf16, kind="ExternalInput"),
        "W4b": nc.dram_tensor("W4b", [128, T, NS * 32], bf16, kind="ExternalInput"),
        "WT16": nc.dram_tensor("WT16", [NS * 32, PEF], f16, kind="ExternalInput"),
        "I128h": nc.dram_tensor("I128h", [128, 128], f16, kind="ExternalInput"),
        "out4": nc.dram_tensor("out4", [NS, D, 128], mybir.dt.float32,
                               kind="ExternalOutput"),
    }
    with tile.TileContext(nc) as tc:
        _emit(tc, nc, bass, mybir, dram)
    nc.compile()
    return nc


def _emit(tc, nc, bass, mybir, dram):
    from contextlib import ExitStack
    f32 = mybir.dt.float32
    f16, bf16 = mybir.dt.float16, mybir.dt.bfloat16
    mult = mybir.AluOpType.mult
    Act = mybir.ActivationFunctionType

    ctx = ExitStack()
    singles = ctx.enter_context(tc.tile_pool(name="singles", bufs=1))
    gpool = ctx.enter_context(tc.tile_pool(name="gth", bufs=2))
    ppool = ctx.enter_context(tc.tile_pool(name="prod", bufs=2))
    small = ctx.enter_context(tc.tile_pool(name="small", bufs=4))
    ps_big = ctx.enter_context(tc.tile_pool(name="psb", bufs=2, space="PSUM"))
    ps_a = ctx.enter_context(tc.tile_pool(name="psa", bufs=2, space="PSUM"))
    ps_s = ctx.enter_context(tc.tile_pool(name="pss", bufs=1, space="PSUM"))
    ps_o = ctx.enter_context(tc.tile_pool(name="pso", bufs=1, space="PSUM"))

    # --- persistent SBUF tensors ---
    xTh = singles.tile([128, T, 128], f16)
    xTb = singles.tile([128, T, 128], bf16)
    W4b = singles.tile([128, T, NS * 32], bf16)
    WT = singles.tile([NS * 32, PEF], f16)
    I128h = singles.tile([128, 128], f16)
    YT4 = singles.tile([128, T, NS * 128], bf16)
    vblk = singles.tile([NS * 32, NS * 128], f16)     # block-diag vsum, [d,b]
    expb = singles.tile([128, PC, NS * 128], bf16)    # exp(logits)
    rd = singles.tile([1, NS * 128], f32)             # 1/den per (n,b)
    vsums = [singles.tile([16, 128], f32, name=f"vsum{i}") for i in range(NS)]
    ones16 = singles.tile([16, 1], f16)
    ones128b = singles.tile([128, 1], bf16)
    eps1 = singles.tile([1, 1], f32)

    nc.sync.dma_start(xTh, dram["xTh"].ap())
    nc.sync.dma_start(xTb, dram["xTb"].ap())
    nc.sync.dma_start(W4b, dram["W4b"].ap())
    nc.sync.dma_start(WT, dram["WT16"].ap())
    nc.sync.dma_start(I128h, dram["I128h"].ap())
    nc.vector.memset(vblk, 0.0)
    nc.vector.memset(ones16, 1.0)
    nc.vector.memset(ones128b, 1.0)
    nc.vector.memset(eps1, EPS)

    xTb_e = xTb.rearrange("p (e c) b -> p e c b", c=PC)   # [128, 8, 9, 128]
    xTh_e = xTh.rearrange("p (e c) b -> p e c b", c=PC)

    def squash_to(n, s_ps, it):
        """s_ps: PSUM [16, 128] unnormalized s for capsule n.
        v -> vsum (+ fp16 vblk slot) on iters 1-2, DMA out on iter 3."""
        t16 = small.tile([16, 128], f32)
        if it == 1:
            nc.vector.tensor_scalar_mul(t16, s_ps, 1.0 / P)
        else:
            rd16 = small.tile([16, 128], f32)
            src = rd[:, n * 128:(n + 1) * 128]
            bc = bass.AP(tensor=src.tensor, offset=src.offset,
                         ap=[list(src.ap[0]), [0, 16]] + list(src.ap[1:]))
            nc.gpsimd.dma_start(out=rd16, in_=bc)
            nc.vector.tensor_mul(t16, s_ps, rd16)
        t2 = small.tile([16, 128], f16)
        nc.vector.tensor_mul(t2, t16, t16)
        sq_ps = ps_s.tile([1, 128], f32)
        nc.tensor.matmul(sq_ps, lhsT=ones16, rhs=t2, start=True, stop=True)
        sqs = small.tile([1, 128], f32)
        nc.vector.tensor_copy(sqs, sq_ps)
        w1 = small.tile([1, 128], f32)
        nc.scalar.activation(w1, sqs, Act.Sqrt, bias=eps1)   # sqrt(sq+eps)
        w2 = small.tile([1, 128], f32)
        nc.vector.tensor_scalar_add(w2, sqs, 1.0)
        nc.vector.tensor_mul(w2, w2, w1)                     # (1+sq)*sqrt
        nc.vector.reciprocal(w1, w2)
        nc.vector.tensor_mul(w1, w1, sqs)                    # squash scale
        sc16 = small.tile([16, 128], f32)
        bc = bass.AP(tensor=w1.tensor, offset=w1.offset,
                     ap=[list(w1.ap[0]), [0, 16]] + list(w1.ap[1:]))
        nc.gpsimd.dma_start(out=sc16, in_=bc)
        if it == 3:
            vout = small.tile([16, 128], f32)
            nc.vector.tensor_mul(vout, t16, sc16)
            nc.sync.dma_start(dram["out4"].ap()[n], vout)
        else:
            slot = vblk[n * 32:n * 32 + 16, n * 128:(n + 1) * 128]
            if it == 1:
                nc.vector.tensor_mul(vsums[n], t16, sc16)
            else:
                vtmp = small.tile([16, 128], f32)
                nc.vector.tensor_mul(vtmp, t16, sc16)
                nc.vector.tensor_add(vsums[n], vsums[n], vtmp)
            nc.vector.tensor_copy(slot, vsums[n])            # fp32 -> fp16

    # ---------- iteration 1: s1 = (1/P) sum_pe x W (all 4 n at once) ----------
    s4w = ps_o.tile([NS * 32, NS * 128], f32, name="s4big")
    s4_ps = s4w[:, :128]
    for t in range(T):
        nc.tensor.matmul(s4_ps, lhsT=W4b[:, t, :], rhs=xTb[:, t, :],
                         start=(t == 0), stop=(t == T - 1))
    for n in range(NS):
        squash_to(n, s4_ps[n * 32:n * 32 + 16, :], 1)

    # ---------- iterations 2, 3 ----------
    for it in (2, 3):
        # logits: G (fp16 MM) -> prod (fp16 DVE) -> e-sum (identity MMs)
        for pc in range(PC):
            aT_ps = ps_a.tile([128, NS * 128], f32)
            gt = gpool.tile([128, E, NS * 128], f16)
            for e in range(E):
                t_idx = e * PC + pc
                g_ps = ps_big.tile([128, NS * 128], f32)
                nc.tensor.matmul(g_ps,
                                 lhsT=WT[:, t_idx * 128:(t_idx + 1) * 128],
                                 rhs=vblk, start=True, stop=True)
                nc.scalar.activation(gt[:, e, :], g_ps, Act.Copy)
            prod = ppool.tile([128, E, NS * 128], f16)
            xe = xTh_e[:, :, pc, :]
            xeb = xe[:, :, None, :].to_broadcast([128, E, NS, 128])
            nc.vector.tensor_tensor(
                prod.rearrange("p e (n b) -> p e n b", n=NS),
                gt.rearrange("p e (n b) -> p e n b", n=NS), xeb, mult)
            for e in range(E):
                nc.tensor.matmul(aT_ps, lhsT=I128h, rhs=prod[:, e, :],
                                 start=(e == 0), stop=(e == E - 1),
                                 skip_group_check=True)
            nc.scalar.activation(expb[:, pc, :], aT_ps, Act.Exp)
        # denominators: sum over p (partitions) via ones matmul
        den_ps = ps_s.tile([1, NS * 128], f32)
        for pc in range(PC):
            nc.tensor.matmul(den_ps, lhsT=ones128b, rhs=expb[:, pc, :],
                             start=(pc == 0), stop=(pc == PC - 1))
        nc.vector.reciprocal(rd, den_ps)
        # Y = exp * x (bf16) for all capsules, then one s-matmul stream
        for n in range(NS):
            ytn = YT4[:, :, n * 128:(n + 1) * 128]
            en = expb[:, :, n * 128:(n + 1) * 128]
            enb = en[:, None, :, :].to_broadcast([128, E, PC, 128])
            nc.vector.tensor_tensor(
                ytn.rearrange("p (e c) b -> p e c b", c=PC), xTb_e, enb, mult)
        s4_ps2 = ps_o.tile([NS * 32, NS * 128], f32, name="s4big")
        for t in range(T):
            nc.tensor.matmul(s4_ps2, lhsT=W4b[:, t, :], rhs=YT4[:, t, :],
                             start=(t == 0), stop=(t == T - 1))
        for n in range(NS):
            squash_to(n, s4_ps2[n * 32:n * 32 + 16, n * 128:(n + 1) * 128], it)
    ctx.close()


def _host_prep(x, W):
    """Per-core input arrays (layout-only transforms)."""
    xT = np.ascontiguousarray(x.transpose(2, 1, 0)).reshape(PEF, B)  # (e,p),b
    xT_t = np.ascontiguousarray(xT.reshape(T, 128, B).transpose(1, 0, 2))
    xTh = xT_t.astype(np.float16)
    xTb = xT_t.astype(ml_dtypes.bfloat16)
    ident = np.eye(128, dtype=np.float16)
    maps = []
    for r in range(NCORES):
        Ws = W[r * NS:(r + 1) * NS]                      # [4, P, E, D]
        Wp = np.zeros((NS, P, E, 32), np.float32)        # pad d 16->32
        Wp[:, :, :, :D] = Ws
        W4 = Wp.transpose(2, 1, 0, 3).reshape(PEF, NS * 32)  # [(e,p),(n,dpad)]
        W4b = np.ascontiguousarray(
            W4.reshape(T, 128, NS * 32).transpose(1, 0, 2)).astype(ml_dtypes.bfloat16)
        WT16 = np.ascontiguousarray(
            Wp.transpose(0, 3, 2, 1).reshape(NS * 32, PEF)).astype(np.float16)
        maps.append({"xTh": xTh, "xTb": xTb, "W4b": W4b,
                     "WT16": WT16, "I128h": ident})
    return maps


def kernel(x, W):
    global _COMPILED
    from concourse import bass_utils
    if _COMPILED is None:
        _COMPILED = _build()
    in_maps = _host_prep(np.asarray(x, np.float32), np.asarray(W, np.float32))
    res = bass_utils.run_bass_kernel_spmd(
        _COMPILED, in_maps, core_ids=list(range(NCORES)))
    out = np.empty((B, N, D), np.float32)
    for r in range(NCORES):
        o = res.results[r]["out4"]                       # [4, 16, 128]
        out[:, r * NS:(r + 1) * NS, :] = np.asarray(o).transpose(2, 0, 1)
    return out


# revision 13
# speedup vs baseline: 1.1028x; 1.1028x over previous
"""DigitalCapsule dynamic-routing kernel for 8 TRN2 NeuronCores.

Math (per batch b, out-capsule n):
    u_hat[p,d] = sum_e x[b,p,e] W[n,p,e,d]
    3 routing iters: c = softmax_p(logits), s = sum_p c*u_hat,
    v = squash(s), logits += v . u_hat
Output v: [B, N, D],  B=128, N=32, P=1152, E=8, D=16.

Sharding: N across the 8 cores (4 capsules each), full B per core.
Routing is independent per n, so there is no cross-core communication.

Formulation avoids materializing u_hat (302 MB) entirely:
  logits:  G[pe,(n,b)] = sum_d W[pe,d] vsum[n,d,b]   (fp16 matmul)
           aT[p,(n,b)] = sum_e xT[pe,b] * G[pe,(n,b)] (fp16 mul on DVE,
                          e-sum via identity matmuls into fp32 PSUM)
           where vsum accumulates v over iters, so logits never need
           to be carried: logits_3 = x.W.(v1+v2).
  s-step:  Y[pe,(n,b)] = exp(aT)*xT (bf16);  s = sum_pe Y W  (one
           free-512 matmul stream covers all 4 capsules).
All contractions run on the tensor engine; PSUM accumulates in fp32.
v lives in [d, b] layout throughout - no transposes anywhere.
"""

import numpy as np
import ml_dtypes

B, N, P, E, D = 128, 32, 1152, 8, 16
NCORES = 8
NS = N // NCORES          # capsules per core
PEF = P * E               # 9216 flattened (e, p) contraction dim
T = PEF // 128            # 72 K-tiles
PC = P // 128             # 9 p-chunks
EPS = 1e-8

_COMPILED = None


def _build():
    import concourse.bass as bass
    import concourse.tile as tile
    from concourse import bacc, mybir

    nc = bacc.Bacc("TRN2", target_bir_lowering=False)
    f16, bf16 = mybir.dt.float16, mybir.dt.bfloat16

    dram = {
        "xTh": nc.dram_tensor("xTh", [128, T, 128], f16, kind="ExternalInput"),
        "xTb": nc.dram_tensor("xTb", [128, T, 128], bf16, kind="ExternalInput"),
        "W4b": nc.dram_tensor("W4b", [128, T, NS * 32], bf16, kind="ExternalInput"),
        "WT16": nc.dram_tensor("WT16", [NS * 32, PEF], f16, kind="ExternalInput"),
        "I128h": nc.dram_tensor("I128h", [128, 128], f16, kind="ExternalInput"),
        "out4": nc.dram_tensor("out4", [NS, D, 128], mybir.dt.float32,
                               kind="ExternalOutput"),
    }
    with tile.TileContext(nc) as tc:
        _emit(tc, nc, bass, mybir, dram)
    nc.compile()
    return nc


def _emit(tc, nc, bass, mybir, dram):
    from contextlib import ExitStack
    f32 = mybir.dt.float32
    f16, bf16 = mybir.dt.float16, mybir.dt.bfloat16
    mult = mybir.AluOpType.mult
    Act = mybir.ActivationFunctionType

    ctx = ExitStack()
    singles = ctx.enter_context(tc.tile_pool(name="singles", bufs=1))
    gpool = ctx.enter_context(tc.tile_pool(name="gth", bufs=2))
    ppool = ctx.enter_context(tc.tile_pool(name="prod", bufs=2))
    small = ctx.enter_context(tc.tile_pool(name="small", bufs=4))
    ps_big = ctx.enter_context(tc.tile_pool(name="psb", bufs=3, space="PSUM"))
    ps_a = ctx.enter_context(tc.tile_pool(name="psa", bufs=2, space="PSUM"))
    ps_s = ctx.enter_context(tc.tile_pool(name="pss", bufs=1, space="PSUM"))
    ps_o = ctx.enter_context(tc.tile_pool(name="pso", bufs=1, space="PSUM"))

    # --- persistent SBUF tensors ---
    xTh = singles.tile([128, T, 128], f16)
    xTb = singles.tile([128, T, 128], bf16)
    W4b = singles.tile([128, T, NS * 32], bf16)
    WT = singles.tile([NS * 32, PEF], f16)
    I128h = singles.tile([128, 128], f16)
    YT4 = singles.tile([128, T, NS * 128], bf16)
    vblk = singles.tile([NS * 32, NS * 128], f16)     # block-diag vsum, [d,b]
    expb = singles.tile([128, PC, NS * 128], bf16)    # exp(logits)
    rd = singles.tile([1, NS * 128], f32)             # 1/den per (n,b)
    vsums = [singles.tile([16, 128], f32, name=f"vsum{i}") for i in range(NS)]
    ones16 = singles.tile([16, 1], f16)
    ones128b = singles.tile([128, 1], bf16)
    eps1 = singles.tile([1, 1], f32)

    nc.sync.dma_start(xTh, dram["xTh"].ap())
    nc.sync.dma_start(xTb, dram["xTb"].ap())
    nc.sync.dma_start(W4b, dram["W4b"].ap())
    nc.sync.dma_start(WT, dram["WT16"].ap())
    nc.sync.dma_start(I128h, dram["I128h"].ap())
    nc.vector.memset(vblk, 0.0)
    nc.vector.memset(ones16, 1.0)
    nc.vector.memset(ones128b, 1.0)
    nc.vector.memset(eps1, EPS)

    xTb_e = xTb.rearrange("p (e c) b -> p e c b", c=PC)   # [128, 8, 9, 128]
    xTh_e = xTh.rearrange("p (e c) b -> p e c b", c=PC)

    def squash_to(n, s_ps, it):
        """s_ps: PSUM [16, 128] unnormalized s for capsule n.
        v -> vsum (+ fp16 vblk slot) on iters 1-2, DMA out on iter 3."""
        t16 = small.tile([16, 128], f32)
        if it == 1:
            nc.vector.tensor_scalar_mul(t16, s_ps, 1.0 / P)
        else:
            rd16 = small.tile([16, 128], f32)
            src = rd[:, n * 128:(n + 1) * 128]
            bc = bass.AP(tensor=src.tensor, offset=src.offset,
                         ap=[list(src.ap[0]), [0, 16]] + list(src.ap[1:]))
            nc.gpsimd.dma_start(out=rd16, in_=bc)
            nc.vector.tensor_mul(t16, s_ps, rd16)
        t2 = small.tile([16, 128], f16)
        nc.vector.tensor_mul(t2, t16, t16)
        sq_ps = ps_s.tile([1, 128], f32)
        nc.tensor.matmul(sq_ps, lhsT=ones16, rhs=t2, start=True, stop=True)
        sqs = small.tile([1, 128], f32)
        nc.vector.tensor_copy(sqs, sq_ps)
        w1 = small.tile([1, 128], f32)
        nc.scalar.activation(w1, sqs, Act.Sqrt, bias=eps1)   # sqrt(sq+eps)
        w2 = small.tile([1, 128], f32)
        nc.vector.tensor_scalar_add(w2, sqs, 1.0)
        nc.vector.tensor_mul(w2, w2, w1)                     # (1+sq)*sqrt
        nc.vector.reciprocal(w1, w2)
        nc.vector.tensor_mul(w1, w1, sqs)                    # squash scale
        sc16 = small.tile([16, 128], f32)
        bc = bass.AP(tensor=w1.tensor, offset=w1.offset,
                     ap=[list(w1.ap[0]), [0, 16]] + list(w1.ap[1:]))
        nc.gpsimd.dma_start(out=sc16, in_=bc)
        if it == 3:
            vout = small.tile([16, 128], f32)
            nc.vector.tensor_mul(vout, t16, sc16)
            nc.sync.dma_start(dram["out4"].ap()[n], vout)
        else:
            slot = vblk[n * 32:n * 32 + 16, n * 128:(n + 1) * 128]
            if it == 1:
                nc.vector.tensor_mul(vsums[n], t16, sc16)
            else:
                vtmp = small.tile([16, 128], f32)
                nc.vector.tensor_mul(vtmp, t16, sc16)
                nc.vector.tensor_add(vsums[n], vsums[n], vtmp)
            nc.vector.tensor_copy(slot, vsums[n])            # fp32 -> fp16

    # ---------- iteration 1: s1 = (1/P) sum_pe x W (all 4 n at once) ----------
    s4w = ps_o.tile([NS * 32, NS * 128], f32, name="s4big")
    s4_ps = s4w[:, :128]
    for t in range(T):
        nc.tensor.matmul(s4_ps, lhsT=W4b[:, t, :], rhs=xTb[:, t, :],
                         start=(t == 0), stop=(t == T - 1))
    for n in range(NS):
        squash_to(n, s4_ps[n * 32:n * 32 + 16, :], 1)

    # ---------- iterations 2, 3 ----------
    for it in (2, 3):
        # logits: G (fp16 MM) -> prod (fp16 DVE) -> e-sum (identity MMs)
        for pc in range(PC):
            aT_ps = ps_a.tile([128, NS * 128], f32)
            gt = gpool.tile([128, E, NS * 128], f16)
            prod = ppool.tile([128, E, NS * 128], f16)
            for e in range(E):
                t_idx = e * PC + pc
                g_ps = ps_big.tile([128, NS * 128], f32)
                nc.tensor.matmul(g_ps,
                                 lhsT=WT[:, t_idx * 128:(t_idx + 1) * 128],
                                 rhs=vblk, start=True, stop=True)
                if e % 4 == 3:
                    nc.vector.tensor_copy(gt[:, e, :], g_ps)
                else:
                    nc.scalar.activation(gt[:, e, :], g_ps, Act.Copy)
            xe = xTh_e[:, :, pc, :]
            for h in range(2):
                sl = slice(h * 4, h * 4 + 4)
                xeb = xe[:, sl, None, :].to_broadcast([128, 4, NS, 128])
                nc.vector.tensor_tensor(
                    prod[:, sl].rearrange("p e (n b) -> p e n b", n=NS),
                    gt[:, sl].rearrange("p e (n b) -> p e n b", n=NS),
                    xeb, mult)
            for e in range(E):
                nc.tensor.matmul(aT_ps, lhsT=I128h, rhs=prod[:, e, :],
                                 start=(e == 0), stop=(e == E - 1),
                                 skip_group_check=True)
            nc.scalar.activation(expb[:, pc, :], aT_ps, Act.Exp)
        # denominators: sum over p (partitions) via ones matmul
        den_ps = ps_s.tile([1, NS * 128], f32)
        for pc in range(PC):
            nc.tensor.matmul(den_ps, lhsT=ones128b, rhs=expb[:, pc, :],
                             start=(pc == 0), stop=(pc == PC - 1))
        nc.vector.reciprocal(rd, den_ps)
        # Y = exp * x (bf16) for all capsules, then one s-matmul stream
        for n in range(NS):
            ytn = YT4[:, :, n * 128:(n + 1) * 128]
            en = expb[:, :, n * 128:(n + 1) * 128]
            enb = en[:, None, :, :].to_broadcast([128, E, PC, 128])
            eng = nc.gpsimd if n == 0 else nc.vector
            eng.tensor_tensor(
                ytn.rearrange("p (e c) b -> p e c b", c=PC), xTb_e, enb, mult)
        s4_ps2 = ps_o.tile([NS * 32, NS * 128], f32, name="s4big")
        for t in range(T):
            nc.tensor.matmul(s4_ps2, lhsT=W4b[:, t, :], rhs=YT4[:, t, :],
                             start=(t == 0), stop=(t == T - 1))
        for n in range(NS):
            squash_to(n, s4_ps2[n * 32:n * 32 + 16, n * 128:(n + 1) * 128], it)
    ctx.close()


def _host_prep(x, W):
    """Per-core input arrays (layout-only transforms)."""
    xT = np.ascontiguousarray(x.transpose(2, 1, 0)).reshape(PEF, B)  # (e,p),b
    xT_t = np.ascontiguousarray(xT.reshape(T, 128, B).transpose(1, 0, 2))
    xTh = xT_t.astype(np.float16)
    xTb = xT_t.astype(ml_dtypes.bfloat16)
    ident = np.eye(128, dtype=np.float16)
    maps = []
    for r in range(NCORES):
        Ws = W[r * NS:(r + 1) * NS]                      # [4, P, E, D]
        Wp = np.zeros((NS, P, E, 32), np.float32)        # pad d 16->32
        Wp[:, :, :, :D] = Ws
        W4 = Wp.transpose(2, 1, 0, 3).reshape(PEF, NS * 32)  # [(e,p),(n,dpad)]
        W4b = np.ascontiguousarray(
            W4.reshape(T, 128, NS * 32).transpose(1, 0, 2)).astype(ml_dtypes.bfloat16)
        WT16 = np.ascontiguousarray(
            Wp.transpose(0, 3, 2, 1).reshape(NS * 32, PEF)).astype(np.float16)
        maps.append({"xTh": xTh, "xTb": xTb, "W4b": W4b,
                     "WT16": WT16, "I128h": ident})
    return maps


def kernel(x, W):
    global _COMPILED
    from concourse import bass_utils
    if _COMPILED is None:
        _COMPILED = _build()
    in_maps = _host_prep(np.asarray(x, np.float32), np.asarray(W, np.float32))
    res = bass_utils.run_bass_kernel_spmd(
        _COMPILED, in_maps, core_ids=list(range(NCORES)))
    out = np.empty((B, N, D), np.float32)
    for r in range(NCORES):
        o = res.results[r]["out4"]                       # [4, 16, 128]
        out[:, r * NS:(r + 1) * NS, :] = np.asarray(o).transpose(2, 0, 1)
    return out


# revision 14
# speedup vs baseline: 1.4687x; 1.3318x over previous
"""DigitalCapsule dynamic-routing kernel for 8 TRN2 NeuronCores.

Math (per batch b, out-capsule n):
    u_hat[p,d] = sum_e x[b,p,e] W[n,p,e,d]
    3 routing iters: c = softmax_p(logits), s = sum_p c*u_hat,
    v = squash(s), logits += v . u_hat
Output v: [B, N, D],  B=128, N=32, P=1152, E=8, D=16.

Sharding: N across the 8 cores (4 capsules each), full B per core.
Routing is independent per n, so there is no cross-core communication.

Formulation avoids materializing u_hat (302 MB) entirely:
  logits:  G[pe,(n,b)] = sum_d W[pe,d] vsum[n,d,b]   (fp16 matmul)
           aT[p,(n,b)] = sum_e xT[pe,b] * G[pe,(n,b)] (fp16 mul on DVE,
                          e-sum via identity matmuls into fp32 PSUM)
           where vsum accumulates v over iters, so logits never need
           to be carried: logits_3 = x.W.(v1+v2).
  s-step:  Y[pe,(n,b)] = exp(aT)*xT (bf16);  s = sum_pe Y W  (one
           free-512 matmul stream covers all 4 capsules).
All contractions run on the tensor engine; PSUM accumulates in fp32.
v lives in [d, b] layout throughout - no transposes anywhere.
"""

import numpy as np
import ml_dtypes

B, N, P, E, D = 128, 32, 1152, 8, 16
NCORES = 8
NS = N // NCORES          # capsules per core
PEF = P * E               # 9216 flattened (e, p) contraction dim
T = PEF // 128            # 72 K-tiles
PC = P // 128             # 9 p-chunks
EPS = 1e-8

_COMPILED = None


def _build():
    import concourse.bass as bass
    import concourse.tile as tile
    from concourse import bacc, mybir

    nc = bacc.Bacc("TRN2", target_bir_lowering=False)
    f16, bf16 = mybir.dt.float16, mybir.dt.bfloat16

    dram = {
        "xTh": nc.dram_tensor("xTh", [128, T, 128], f16, kind="ExternalInput"),
        "xTb": nc.dram_tensor("xTb", [128, T, 128], bf16, kind="ExternalInput"),
        "W4b": nc.dram_tensor("W4b", [128, T, NS * 32], bf16, kind="ExternalInput"),
        "WT16": nc.dram_tensor("WT16", [NS * 32, PEF], f16, kind="ExternalInput"),
        "I128h": nc.dram_tensor("I128h", [128, 128], f16, kind="ExternalInput"),
        "out4": nc.dram_tensor("out4", [NS, D, 128], mybir.dt.float32,
                               kind="ExternalOutput"),
    }
    with tile.TileContext(nc) as tc:
        _emit(tc, nc, bass, mybir, dram)
    nc.compile()
    return nc


def _emit(tc, nc, bass, mybir, dram):
    from contextlib import ExitStack
    f32 = mybir.dt.float32
    f16, bf16 = mybir.dt.float16, mybir.dt.bfloat16
    mult = mybir.AluOpType.mult
    Act = mybir.ActivationFunctionType

    ctx = ExitStack()
    singles = ctx.enter_context(tc.tile_pool(name="singles", bufs=1))
    gpool = ctx.enter_context(tc.tile_pool(name="gth", bufs=2))
    ppool = ctx.enter_context(tc.tile_pool(name="prod", bufs=2))
    small = ctx.enter_context(tc.tile_pool(name="small", bufs=4))
    ps_big = ctx.enter_context(tc.tile_pool(name="psb", bufs=3, space="PSUM"))
    ps_a = ctx.enter_context(tc.tile_pool(name="psa", bufs=2, space="PSUM"))
    ps_s = ctx.enter_context(tc.tile_pool(name="pss", bufs=1, space="PSUM"))
    ps_o = ctx.enter_context(tc.tile_pool(name="pso", bufs=1, space="PSUM"))

    # --- persistent SBUF tensors ---
    xTh = singles.tile([128, T, 128], f16)
    xTb = singles.tile([128, T, 128], bf16)
    W4b = singles.tile([128, T, NS * 32], bf16)
    WT = singles.tile([NS * 32, PEF], f16)
    I128h = singles.tile([128, 128], f16)
    YT4 = singles.tile([128, T, NS * 128], bf16)
    vblk = singles.tile([NS * 32, NS * 128], f16)     # block-diag vsum, [d,b]
    expb = singles.tile([128, PC, NS * 128], bf16)    # exp(logits)
    rd = singles.tile([1, NS * 128], f32)             # 1/den per (n,b)
    vsums = [singles.tile([16, 128], f32, name=f"vsum{i}") for i in range(NS)]
    ones16 = singles.tile([16, 1], f16)
    ones128b = singles.tile([128, 1], bf16)
    eps1 = singles.tile([1, 1], f32)

    nc.sync.dma_start(xTb, dram["xTb"].ap())
    nc.sync.dma_start(W4b, dram["W4b"].ap())
    nc.sync.dma_start(I128h, dram["I128h"].ap())
    nc.sync.dma_start(WT, dram["WT16"].ap())
    nc.sync.dma_start(xTh, dram["xTh"].ap())
    nc.vector.memset(vblk, 0.0)
    nc.vector.memset(ones16, 1.0)
    nc.vector.memset(ones128b, 1.0)
    nc.vector.memset(eps1, EPS)

    xTb_e = xTb.rearrange("p (e c) b -> p e c b", c=PC)   # [128, 8, 9, 128]
    xTh_e = xTh.rearrange("p (e c) b -> p e c b", c=PC)

    def squash_to(n, s_ps, it):
        """s_ps: PSUM [16, 128] unnormalized s for capsule n.
        v -> vsum (+ fp16 vblk slot) on iters 1-2, DMA out on iter 3."""
        t16 = small.tile([16, 128], f32)
        if it == 1:
            nc.vector.tensor_scalar_mul(t16, s_ps, 1.0 / P)
        else:
            rd16 = small.tile([16, 128], f32)
            src = rd[:, n * 128:(n + 1) * 128]
            bc = bass.AP(tensor=src.tensor, offset=src.offset,
                         ap=[list(src.ap[0]), [0, 16]] + list(src.ap[1:]))
            nc.gpsimd.dma_start(out=rd16, in_=bc)
            nc.vector.tensor_mul(t16, s_ps, rd16)
        t2 = small.tile([16, 128], f16)
        nc.vector.tensor_mul(t2, t16, t16)
        sq_ps = ps_s.tile([1, 128], f32)
        nc.tensor.matmul(sq_ps, lhsT=ones16, rhs=t2, start=True, stop=True)
        sqs = small.tile([1, 128], f32)
        nc.vector.tensor_copy(sqs, sq_ps)
        w1 = small.tile([1, 128], f32)
        nc.scalar.activation(w1, sqs, Act.Sqrt, bias=eps1)   # sqrt(sq+eps)
        w2 = small.tile([1, 128], f32)
        nc.vector.tensor_scalar_add(w2, sqs, 1.0)
        nc.vector.tensor_mul(w2, w2, w1)                     # (1+sq)*sqrt
        nc.vector.reciprocal(w1, w2)
        nc.vector.tensor_mul(w1, w1, sqs)                    # squash scale
        sc16 = small.tile([16, 128], f32)
        bc = bass.AP(tensor=w1.tensor, offset=w1.offset,
                     ap=[list(w1.ap[0]), [0, 16]] + list(w1.ap[1:]))
        nc.gpsimd.dma_start(out=sc16, in_=bc)
        if it == 3:
            vout = small.tile([16, 128], f32)
            nc.vector.tensor_mul(vout, t16, sc16)
            nc.sync.dma_start(dram["out4"].ap()[n], vout)
        else:
            slot = vblk[n * 32:n * 32 + 16, n * 128:(n + 1) * 128]
            if it == 1:
                nc.vector.tensor_mul(vsums[n], t16, sc16)
            else:
                vtmp = small.tile([16, 128], f32)
                nc.vector.tensor_mul(vtmp, t16, sc16)
                nc.vector.tensor_add(vsums[n], vsums[n], vtmp)
            nc.vector.tensor_copy(slot, vsums[n])            # fp32 -> fp16

    # ---------- iteration 1: s1 = (1/P) sum_pe x W (all 4 n at once) ----------
    s4w = ps_o.tile([NS * 32, NS * 128], f32, name="s4big")
    s4_ps = s4w[:, :128]
    for t in range(T):
        nc.tensor.matmul(s4_ps, lhsT=W4b[:, t, :], rhs=xTb[:, t, :],
                         start=(t == 0), stop=(t == T - 1))
    for n in range(NS):
        squash_to(n, s4_ps[n * 32:n * 32 + 16, :], 1)

    # ---------- iterations 2, 3 ----------
    for it in (2, 3):
        # logits: G (fp16 MM) -> prod (fp16 DVE) -> e-sum (identity MMs)
        for pc in range(PC):
            aT_ps = ps_a.tile([128, NS * 128], f32)
            gt = gpool.tile([128, E, NS * 128], f16)
            prod = ppool.tile([128, E, NS * 128], f16)
            for e in range(E):
                t_idx = e * PC + pc
                g_ps = ps_big.tile([128, NS * 128], f32)
                nc.tensor.matmul(g_ps,
                                 lhsT=WT[:, t_idx * 128:(t_idx + 1) * 128],
                                 rhs=vblk, start=True, stop=True)
                if e % 4 == 3:
                    nc.vector.tensor_copy(gt[:, e, :], g_ps)
                else:
                    nc.scalar.activation(gt[:, e, :], g_ps, Act.Copy)
            xe = xTh_e[:, :, pc, :]
            for h in range(2):
                sl = slice(h * 4, h * 4 + 4)
                xeb = xe[:, sl, None, :].to_broadcast([128, 4, NS, 128])
                nc.vector.tensor_tensor(
                    prod[:, sl].rearrange("p e (n b) -> p e n b", n=NS),
                    gt[:, sl].rearrange("p e (n b) -> p e n b", n=NS),
                    xeb, mult)
            for e in range(E):
                nc.tensor.matmul(aT_ps, lhsT=I128h, rhs=prod[:, e, :],
                                 start=(e == 0), stop=(e == E - 1),
                                 skip_group_check=True)
            nc.scalar.activation(expb[:, pc, :], aT_ps, Act.Exp)
        # denominators: sum over p (partitions) via ones matmul
        den_ps = ps_s.tile([1, NS * 128], f32)
        for pc in range(PC):
            nc.tensor.matmul(den_ps, lhsT=ones128b, rhs=expb[:, pc, :],
                             start=(pc == 0), stop=(pc == PC - 1))
        nc.vector.reciprocal(rd, den_ps)
        # Y = exp * x (bf16), chunked by p-column-group so the s-matmul
        # stream starts while later chunks (and later exp pcs) are in flight
        YT4_e = YT4.rearrange("p (e c) nb -> p e c nb", c=PC)
        s4_ps2 = ps_o.tile([NS * 32, NS * 128], f32, name="s4big")
        CH = 3
        for c0 in range(0, PC, CH):
            for n in range(NS):
                ytn = YT4_e[:, :, c0:c0 + CH, n * 128:(n + 1) * 128]
                en = expb[:, c0:c0 + CH, n * 128:(n + 1) * 128]
                enb = en[:, None, :, :].to_broadcast([128, E, CH, 128])
                nc.vector.tensor_tensor(
                    ytn, xTb_e[:, :, c0:c0 + CH, :], enb, mult)
            for pc in range(c0, c0 + CH):
                for e in range(E):
                    t = e * PC + pc
                    nc.tensor.matmul(s4_ps2, lhsT=W4b[:, t, :], rhs=YT4[:, t, :],
                                     start=(pc == 0 and e == 0),
                                     stop=(pc == PC - 1 and e == E - 1),
                                     skip_group_check=True)
        for n in range(NS):
            squash_to(n, s4_ps2[n * 32:n * 32 + 16, n * 128:(n + 1) * 128], it)
    ctx.close()


def _host_prep(x, W):
    """Per-core input arrays (layout-only transforms)."""
    xT = np.ascontiguousarray(x.transpose(2, 1, 0)).reshape(PEF, B)  # (e,p),b
    xT_t = np.ascontiguousarray(xT.reshape(T, 128, B).transpose(1, 0, 2))
    xTh = xT_t.astype(np.float16)
    xTb = xT_t.astype(ml_dtypes.bfloat16)
    ident = np.eye(128, dtype=np.float16)
    maps = []
    for r in range(NCORES):
        Ws = W[r * NS:(r + 1) * NS]                      # [4, P, E, D]
        Wp = np.zeros((NS, P, E, 32), np.float32)        # pad d 16->32
        Wp[:, :, :, :D] = Ws
        W4 = Wp.transpose(2, 1, 0, 3).reshape(PEF, NS * 32)  # [(e,p),(n,dpad)]
        W4b = np.ascontiguousarray(
            W4.reshape(T, 128, NS * 32).transpose(1, 0, 2)).astype(ml_dtypes.bfloat16)
        WT16 = np.ascontiguousarray(
            Wp.transpose(0, 3, 2, 1).reshape(NS * 32, PEF)).astype(np.float16)
        maps.append({"xTh": xTh, "xTb": xTb, "W4b": W4b,
                     "WT16": WT16, "I128h": ident})
    return maps


def kernel(x, W):
    global _COMPILED
    from concourse import bass_utils
    if _COMPILED is None:
        _COMPILED = _build()
    in_maps = _host_prep(np.asarray(x, np.float32), np.asarray(W, np.float32))
    res = bass_utils.run_bass_kernel_spmd(
        _COMPILED, in_maps, core_ids=list(range(NCORES)))
    out = np.empty((B, N, D), np.float32)
    for r in range(NCORES):
        o = res.results[r]["out4"]                       # [4, 16, 128]
        out[:, r * NS:(r + 1) * NS, :] = np.asarray(o).transpose(2, 0, 1)
    return out
